# revision 6
# baseline (speedup 1.0000x reference)
"""Trainium2 Bass kernel for nn_CrossAttention (3-head cross-attention + ReLU projection).

Sharding: data-parallel over batch. B=8 -> one batch element per NeuronCore,
identical SPMD program, no collectives. Full inputs in, full output out.

Per-core dataflow (all matmuls in fp32r: bf16 speed at free-dim>=256, ~1.5e-4 err):
  t1,t2 [1024,768]  --PE transpose-->  t1T,t2T [768,1024]
  per head h:
    qT = Wq_h^T-stationary matmuls over t1T (+bq fused in ACT eviction)   [768,1024]
    kT = same from t2T (+bk)                                              [768,1024]
    V  = t2T-stationary matmuls with Wv_h (+bv broadcast fused in DVE)    [1024,768]
    per q-tile (128 rows):
      S = qT^T @ kT  (PSUM, fp32)                [128,1024]
      softmax: reduce_max -> Exp(bias=-max, accum_out=rowsum) -> P (f32r, unnormalized)
      PT = PE-transpose(P)                       [1024,128]
      C = PT^T @ V  (PSUM)                       [128,768]
      multi = Relu(C * 1/rowsum)  (ACT eviction) -> PE transpose -> DRAM stash
  out = multiT^T-stationary @ Wp (+bp broadcast fused in DVE eviction)    [1024,768]
"""
import numpy as np

import concourse.bass as bass
import concourse.mybir as mybir
from concourse import bacc
from concourse.tile import TileContext

F32 = mybir.dt.float32
F32R = mybir.dt.float32r
AF = mybir.ActivationFunctionType
AX = mybir.AxisListType
ALU = mybir.AluOpType

L = 1024          # LQ = LK
H = 768           # H1 = H2
NH = 3            # heads
HC = H // 128     # 6 chunks of the hidden dim
LC = L // 128     # 8 chunks of the seq dim
QT = L // 128     # 8 q-tiles
HE = NH * H       # 2304 concat dim
HET = HE // 128   # 18 chunks

_CACHE = {}


def build():
    nc = bacc.Bacc()
    t1 = nc.declare_dram_parameter("t1", [L, H], F32R, isOutput=False)
    t2 = nc.declare_dram_parameter("t2", [L, H], F32R, isOutput=False)
    wq = nc.declare_dram_parameter("wq", [NH, H, H], F32R, isOutput=False)
    wk = nc.declare_dram_parameter("wk", [NH, H, H], F32R, isOutput=False)
    wv = nc.declare_dram_parameter("wv", [NH, H, H], F32R, isOutput=False)
    wp = nc.declare_dram_parameter("wp", [HE, H], F32R, isOutput=False)
    bq_sb = nc.declare_dram_parameter("bq_sb", [NH, 128, HC], F32, isOutput=False)
    bk_sb = nc.declare_dram_parameter("bk_sb", [NH, 128, HC], F32, isOutput=False)
    bv_bc = nc.declare_dram_parameter("bv_bc", [NH, 128, H], F32, isOutput=False)
    bp_bc = nc.declare_dram_parameter("bp_bc", [128, H], F32, isOutput=False)
    ident_d = nc.declare_dram_parameter("ident", [128, 128], F32R, isOutput=False)
    out_d = nc.declare_dram_parameter("out", [L, H], F32, isOutput=True)

    # DRAM stash for transposed relu(ctx): [head, e_chunk, 128, L]
    mstash = nc.dram_tensor("mstash", [NH, HC, 128, L], F32R)

    with TileContext(nc) as tc:
        with tc.tile_pool(name="psA", bufs=2, space="PSUM") as psA, \
             tc.tile_pool(name="psB", bufs=2, space="PSUM") as psB, \
             tc.tile_pool(name="psT", bufs=2, space="PSUM") as psT, \
             tc.tile_pool(name="small", bufs=1) as small:

            ident = small.tile([128, 128], F32R, name="ident")
            nc.sync.dma_start(out=ident[:], in_=ident_d[:])
            bp_t = small.tile([128, H], F32, name="bp_t")
            nc.sync.dma_start(out=bp_t[:], in_=bp_bc[:])

            with tc.tile_pool(name="pers", bufs=1) as pers, \
                 tc.tile_pool(name="wpool", bufs=8) as wpool, \
                 tc.tile_pool(name="hb", bufs=2) as hb, \
                 tc.tile_pool(name="work", bufs=2) as work, \
                 tc.tile_pool(name="stats", bufs=4) as stats:

                # ---------- Phase 0: load + transpose inputs ----------
                t1T = pers.tile([128, HC * L], F32R, name="t1T")
                t2T = pers.tile([128, HC * L], F32R, name="t2T")
                for src, dstT in ((t1, t1T), (t2, t2T)):
                    for c in range(LC):
                        nat = work.tile([128, H], F32R, name="nat", tag="nat", bufs=3)
                        nc.sync.dma_start(out=nat[:], in_=src[c * 128:(c + 1) * 128, :])
                        for d in range(HC):
                            pt = psT.tile([128, 128], F32R, tag="tr")
                            nc.tensor.transpose(pt[:], nat[:, d * 128:(d + 1) * 128], ident[:])
                            nc.vector.tensor_copy(
                                dstT[:, d * L + c * 128: d * L + (c + 1) * 128], pt[:])

                qTt = pers.tile([128, HC * L], F32R, name="qTt")
                kTt = pers.tile([128, HC * L], F32R, name="kTt")
                Vt = pers.tile([128, LC * H], F32R, name="Vt")

                for h in range(NH):
                    # ---------- Phase 1: projections for head h ----------
                    bqs = hb.tile([128, HC], F32, name="bqs", tag="bqs")
                    nc.sync.dma_start(out=bqs[:], in_=bq_sb[h])
                    bks = hb.tile([128, HC], F32, name="bks", tag="bks")
                    nc.sync.dma_start(out=bks[:], in_=bk_sb[h])
                    bvb = hb.tile([128, H], F32, name="bvb", tag="bvb")
                    nc.sync.dma_start(out=bvb[:], in_=bv_bc[h])

                    for (wsrc, srcT, dstT, bias) in ((wq, t1T, qTt, bqs), (wk, t2T, kTt, bks)):
                        wch = []
                        for d in range(HC):
                            wt = wpool.tile([128, H], F32R, name="w", tag="w")
                            nc.sync.dma_start(out=wt[:], in_=wsrc[h, d * 128:(d + 1) * 128, :])
                            wch.append(wt)
                        for e in range(HC):
                            for qh in range(2):
                                ps = psB.tile([128, 512], F32, tag="b")
                                for d in range(HC):
                                    nc.tensor.matmul(
                                        ps[:],
                                        wch[d][:, e * 128:(e + 1) * 128],
                                        srcT[:, d * L + qh * 512: d * L + (qh + 1) * 512],
                                        start=(d == 0), stop=(d == HC - 1))
                                nc.scalar.activation(
                                    dstT[:, e * L + qh * 512: e * L + (qh + 1) * 512],
                                    ps[:], AF.Identity, bias=bias[:, e:e + 1], scale=1.0)

                    wch = []
                    for d in range(HC):
                        wt = wpool.tile([128, H], F32R, name="w", tag="w")
                        nc.sync.dma_start(out=wt[:], in_=wv[h, d * 128:(d + 1) * 128, :])
                        wch.append(wt)
                    for kc in range(LC):
                        for (n0, nw) in ((0, 512), (512, 256)):
                            ps = psB.tile([128, nw], F32, tag="b")
                            for d in range(HC):
                                nc.tensor.matmul(
                                    ps[:],
                                    t2T[:, d * L + kc * 128: d * L + (kc + 1) * 128],
                                    wch[d][:, n0:n0 + nw],
                                    start=(d == 0), stop=(d == HC - 1))
                            nc.vector.tensor_add(
                                Vt[:, kc * H + n0: kc * H + n0 + nw],
                                ps[:], bvb[:, n0:n0 + nw])

                    # ---------- Phase 2: attention per q-tile ----------
                    for qt in range(QT):
                        s_ps = psA.tile([128, 1024], F32, tag="s")
                        for nh2 in range(2):
                            for e in range(HC):
                                nc.tensor.matmul(
                                    s_ps[:, nh2 * 512:(nh2 + 1) * 512],
                                    qTt[:, e * L + qt * 128: e * L + (qt + 1) * 128],
                                    kTt[:, e * L + nh2 * 512: e * L + (nh2 + 1) * 512],
                                    start=(e == 0), stop=(e == HC - 1))
                        negmax = stats.tile([128, 1], F32, tag="nm")
                        nc.vector.tensor_reduce(negmax[:], s_ps[:], axis=AX.X,
                                                op=ALU.max, negate=True)
                        P = work.tile([128, 1024], F32R, name="P", tag="P")
                        esum = stats.tile([128, 1], F32, tag="es")
                        nc.scalar.activation(P[:], s_ps[:], AF.Exp,
                                             bias=negmax[:], scale=1.0, accum_out=esum[:])
                        rsum = stats.tile([128, 1], F32, tag="rs")
                        nc.vector.reciprocal(rsum[:], esum[:])

                        PT = work.tile([128, 1024], F32R, name="PT", tag="PT")
                        for kc in range(LC):
                            pt = psT.tile([128, 128], F32R, tag="tr")
                            nc.tensor.transpose(pt[:], P[:, kc * 128:(kc + 1) * 128], ident[:])
                            nc.vector.tensor_copy(PT[:, kc * 128:(kc + 1) * 128], pt[:])

                        mnat = work.tile([128, H], F32R, name="mnat", tag="mnat")
                        for (n0, nw) in ((0, 512), (512, 256)):
                            c_ps = psB.tile([128, nw], F32, tag="b")
                            for kc in range(LC):
                                nc.tensor.matmul(
                                    c_ps[:],
                                    PT[:, kc * 128:(kc + 1) * 128],
                                    Vt[:, kc * H + n0: kc * H + n0 + nw],
                                    start=(kc == 0), stop=(kc == LC - 1))
                            nc.scalar.activation(mnat[:, n0:n0 + nw], c_ps[:],
                                                 AF.Relu, bias=0.0, scale=rsum[:])

                        mt = work.tile([128, H], F32R, name="mt", tag="mt")
                        for e in range(HC):
                            pt = psT.tile([128, 128], F32R, tag="tr")
                            nc.tensor.transpose(pt[:], mnat[:, e * 128:(e + 1) * 128], ident[:])
                            nc.vector.tensor_copy(mt[:, e * 128:(e + 1) * 128], pt[:])
                        nc.sync.dma_start(
                            out=mstash[h, :, :, qt * 128:(qt + 1) * 128]
                                .rearrange("c p q -> p c q"),
                            in_=mt[:].rearrange("p (c q) -> p c q", q=128))

            # ---------- Phase 3: output projection ----------
            with tc.tile_pool(name="wp_pool", bufs=1) as wp_pool, \
                 tc.tile_pool(name="proj", bufs=2) as proj:
                wpt = wp_pool.tile([128, HET * H], F32R, name="wpt")
                for c in range(HET):
                    nc.sync.dma_start(out=wpt[:, c * H:(c + 1) * H],
                                      in_=wp[c * 128:(c + 1) * 128, :])
                for qt in range(QT):
                    mi = proj.tile([128, HET * 128], F32R, name="mi", tag="mi")
                    nc.sync.dma_start(
                        out=mi[:].rearrange("p (h c q) -> p h c q", c=HC, q=128),
                        in_=mstash[:, :, :, qt * 128:(qt + 1) * 128]
                            .rearrange("h c p q -> p h c q"))
                    ob = proj.tile([128, H], F32, name="ob", tag="ob")
                    for (n0, nw) in ((0, 512), (512, 256)):
                        o_ps = psB.tile([128, nw], F32, tag="b")
                        for c in range(HET):
                            nc.tensor.matmul(
                                o_ps[:],
                                mi[:, c * 128:(c + 1) * 128],
                                wpt[:, c * H + n0: c * H + n0 + nw],
                                start=(c == 0), stop=(c == HET - 1))
                        nc.vector.tensor_add(ob[:, n0:n0 + nw], o_ps[:],
                                             bp_t[:, n0:n0 + nw])
                    nc.sync.dma_start(out=out_d[qt * 128:(qt + 1) * 128, :], in_=ob[:])

    nc.finalize()
    return nc


def kernel(tensor1, tensor2, Wq, bq, Wk, bk, Wv, bv, Wp, bp):
    from concourse.bass_utils import run_bass_kernel_spmd

    B = tensor1.shape[0]
    assert B == 8
    if "nc" not in _CACHE:
        _CACHE["nc"] = build()
    nc = _CACHE["nc"]

    f32 = np.float32
    shared = {
        "wq": np.ascontiguousarray(Wq, dtype=f32),
        "wk": np.ascontiguousarray(Wk, dtype=f32),
        "wv": np.ascontiguousarray(Wv, dtype=f32),
        "wp": np.ascontiguousarray(Wp, dtype=f32),
        "bq_sb": np.ascontiguousarray(
            bq.reshape(NH, HC, 128).transpose(0, 2, 1), dtype=f32),
        "bk_sb": np.ascontiguousarray(
            bk.reshape(NH, HC, 128).transpose(0, 2, 1), dtype=f32),
        "bv_bc": np.ascontiguousarray(
            np.broadcast_to(np.asarray(bv, dtype=f32)[:, None, :], (NH, 128, H))),
        "bp_bc": np.ascontiguousarray(
            np.broadcast_to(np.asarray(bp, dtype=f32)[None, :], (128, H))),
        "ident": np.eye(128, dtype=f32),
    }
    in_maps = [
        dict(shared,
             t1=np.ascontiguousarray(tensor1[b], dtype=f32),
             t2=np.ascontiguousarray(tensor2[b], dtype=f32))
        for b in range(B)
    ]
    res = run_bass_kernel_spmd(nc, in_maps, list(range(B)))
    return np.stack([res.results[b]["out"] for b in range(B)], axis=0)


# revision 7
# speedup vs baseline: 1.0466x; 1.0466x over previous
"""Trainium2 Bass kernel for nn_CrossAttention (3-head cross-attention + ReLU projection).

Sharding: data-parallel over batch. B=8 -> one batch element per NeuronCore,
identical SPMD program, no collectives. Full inputs in, full output out.

Per-core dataflow (all matmuls in fp32r: bf16 speed at free-dim>=256, ~1.5e-4 err):
  t1,t2 [1024,768]  --PE transpose-->  t1T,t2T [768,1024]
  per head h:
    qT = Wq_h^T-stationary matmuls over t1T (+bq fused in ACT eviction)   [768,1024]
    kT = same from t2T (+bk)                                              [768,1024]
    V  = t2T-stationary matmuls with Wv_h (+bv broadcast fused in DVE)    [1024,768]
    per q-tile (128 rows):
      S = qT^T @ kT  (PSUM, fp32)                [128,1024]
      softmax: reduce_max -> Exp(bias=-max, accum_out=rowsum) -> P (f32r, unnormalized)
      PT = PE-transpose(P)                       [1024,128]
      C = PT^T @ V  (PSUM)                       [128,768]
      multi = Relu(C * 1/rowsum)  (ACT eviction) -> PE transpose -> DRAM stash
  out = multiT^T-stationary @ Wp (+bp broadcast fused in DVE eviction)    [1024,768]
"""
import numpy as np

import concourse.bass as bass
import concourse.mybir as mybir
from concourse import bacc
from concourse.tile import TileContext

F32 = mybir.dt.float32
F32R = mybir.dt.float32r
AF = mybir.ActivationFunctionType
AX = mybir.AxisListType
ALU = mybir.AluOpType

L = 1024          # LQ = LK
H = 768           # H1 = H2
NH = 3            # heads
HC = H // 128     # 6 chunks of the hidden dim
LC = L // 128     # 8 chunks of the seq dim
QT = L // 128     # 8 q-tiles
HE = NH * H       # 2304 concat dim
HET = HE // 128   # 18 chunks

_CACHE = {}


def build():
    nc = bacc.Bacc()
    t1 = nc.declare_dram_parameter("t1", [L, H], F32R, isOutput=False)
    t2 = nc.declare_dram_parameter("t2", [L, H], F32R, isOutput=False)
    wq = nc.declare_dram_parameter("wq", [NH, H, H], F32R, isOutput=False)
    wk = nc.declare_dram_parameter("wk", [NH, H, H], F32R, isOutput=False)
    wv = nc.declare_dram_parameter("wv", [NH, H, H], F32R, isOutput=False)
    wp = nc.declare_dram_parameter("wp", [HE, H], F32R, isOutput=False)
    bq_sb = nc.declare_dram_parameter("bq_sb", [NH, 128, HC], F32, isOutput=False)
    bk_sb = nc.declare_dram_parameter("bk_sb", [NH, 128, HC], F32, isOutput=False)
    bv_bc = nc.declare_dram_parameter("bv_bc", [NH, 128, H], F32, isOutput=False)
    bp_bc = nc.declare_dram_parameter("bp_bc", [128, H], F32, isOutput=False)
    ident_d = nc.declare_dram_parameter("ident", [128, 128], F32R, isOutput=False)
    out_d = nc.declare_dram_parameter("out", [L, H], F32, isOutput=True)

    # DRAM stash for transposed relu(ctx): [head, e_chunk, 128, L]
    mstash = nc.dram_tensor("mstash", [NH, HC, 128, L], F32R)

    with TileContext(nc) as tc:
        with tc.tile_pool(name="psA", bufs=2, space="PSUM") as psA, \
             tc.tile_pool(name="psB", bufs=2, space="PSUM") as psB, \
             tc.tile_pool(name="psT", bufs=2, space="PSUM") as psT, \
             tc.tile_pool(name="small", bufs=1) as small, \
             tc.tile_pool(name="pers", bufs=1) as pers, \
             tc.tile_pool(name="work", bufs=2) as work, \
             tc.tile_pool(name="hb", bufs=2) as hb, \
             tc.tile_pool(name="stats", bufs=4) as stats:

            ident = small.tile([128, 128], F32R, name="ident")
            nc.sync.dma_start(out=ident[:], in_=ident_d[:])
            bp_t = small.tile([128, H], F32, name="bp_t")
            nc.sync.dma_start(out=bp_t[:], in_=bp_bc[:])

            qTt = pers.tile([128, HC * L], F32R, name="qTt")
            kTt = pers.tile([128, HC * L], F32R, name="kTt")
            Vt = pers.tile([128, LC * H], F32R, name="Vt")

            def projections(h, t1T, t2T, wpool):
                bqs = hb.tile([128, HC], F32, name="bqs", tag="bqs")
                nc.sync.dma_start(out=bqs[:], in_=bq_sb[h])
                bks = hb.tile([128, HC], F32, name="bks", tag="bks")
                nc.sync.dma_start(out=bks[:], in_=bk_sb[h])
                bvb = hb.tile([128, H], F32, name="bvb", tag="bvb")
                nc.sync.dma_start(out=bvb[:], in_=bv_bc[h])

                for (wsrc, srcT, dstT, bias) in ((wq, t1T, qTt, bqs), (wk, t2T, kTt, bks)):
                    wch = []
                    for d in range(HC):
                        wt = wpool.tile([128, H], F32R, name="w", tag="w")
                        nc.sync.dma_start(out=wt[:], in_=wsrc[h, d * 128:(d + 1) * 128, :])
                        wch.append(wt)
                    for e in range(HC):
                        for qh in range(2):
                            ps = psB.tile([128, 512], F32, tag="b")
                            for d in range(HC):
                                nc.tensor.matmul(
                                    ps[:],
                                    wch[d][:, e * 128:(e + 1) * 128],
                                    srcT[:, d * L + qh * 512: d * L + (qh + 1) * 512],
                                    start=(d == 0), stop=(d == HC - 1))
                            nc.scalar.activation(
                                dstT[:, e * L + qh * 512: e * L + (qh + 1) * 512],
                                ps[:], AF.Identity, bias=bias[:, e:e + 1], scale=1.0)

                wch = []
                for d in range(HC):
                    wt = wpool.tile([128, H], F32R, name="w", tag="w")
                    nc.sync.dma_start(out=wt[:], in_=wv[h, d * 128:(d + 1) * 128, :])
                    wch.append(wt)
                for kc in range(LC):
                    for (n0, nw) in ((0, 512), (512, 256)):
                        ps = psB.tile([128, nw], F32, tag="b")
                        for d in range(HC):
                            nc.tensor.matmul(
                                ps[:],
                                t2T[:, d * L + kc * 128: d * L + (kc + 1) * 128],
                                wch[d][:, n0:n0 + nw],
                                start=(d == 0), stop=(d == HC - 1))
                        nc.vector.tensor_add(
                            Vt[:, kc * H + n0: kc * H + n0 + nw],
                            ps[:], bvb[:, n0:n0 + nw])

            def attention(h):
                for qt in range(QT):
                    s_ps = psA.tile([128, 1024], F32, tag="s")
                    for nh2 in range(2):
                        for e in range(HC):
                            nc.tensor.matmul(
                                s_ps[:, nh2 * 512:(nh2 + 1) * 512],
                                qTt[:, e * L + qt * 128: e * L + (qt + 1) * 128],
                                kTt[:, e * L + nh2 * 512: e * L + (nh2 + 1) * 512],
                                start=(e == 0), stop=(e == HC - 1))
                    negmax = stats.tile([128, 1], F32, tag="nm")
                    nc.vector.tensor_reduce(negmax[:], s_ps[:], axis=AX.X,
                                            op=ALU.max, negate=True)
                    P = work.tile([128, 1024], F32R, name="P", tag="P")
                    esum = stats.tile([128, 1], F32, tag="es")
                    nc.scalar.activation(P[:], s_ps[:], AF.Exp,
                                         bias=negmax[:], scale=1.0, accum_out=esum[:])
                    rsum = stats.tile([128, 1], F32, tag="rs")
                    nc.vector.reciprocal(rsum[:], esum[:])

                    PT = work.tile([128, 1024], F32R, name="PT", tag="PT")
                    for kc in range(LC):
                        pt = psT.tile([128, 128], F32R, tag="tr")
                        nc.tensor.transpose(pt[:], P[:, kc * 128:(kc + 1) * 128], ident[:])
                        nc.vector.tensor_copy(PT[:, kc * 128:(kc + 1) * 128], pt[:])

                    mnat = work.tile([128, H], F32R, name="mnat", tag="mnat")
                    for (n0, nw) in ((0, 512), (512, 256)):
                        c_ps = psB.tile([128, nw], F32, tag="b")
                        for kc in range(LC):
                            nc.tensor.matmul(
                                c_ps[:],
                                PT[:, kc * 128:(kc + 1) * 128],
                                Vt[:, kc * H + n0: kc * H + n0 + nw],
                                start=(kc == 0), stop=(kc == LC - 1))
                        nc.scalar.activation(mnat[:, n0:n0 + nw], c_ps[:],
                                             AF.Relu, bias=0.0, scale=rsum[:])

                    mt = work.tile([128, H], F32R, name="mt", tag="mt")
                    for e in range(HC):
                        pt = psT.tile([128, 128], F32R, tag="tr")
                        nc.tensor.transpose(pt[:], mnat[:, e * 128:(e + 1) * 128], ident[:])
                        nc.vector.tensor_copy(mt[:, e * 128:(e + 1) * 128], pt[:])
                    nc.sync.dma_start(
                        out=mstash[h, :, :, qt * 128:(qt + 1) * 128]
                            .rearrange("c p q -> p c q"),
                        in_=mt[:].rearrange("p (c q) -> p c q", q=128))

            # Input transposes + all projections + attention for heads 0,1 live
            # inside the inputs/weights pool scope; the scope closes right
            # before head 2's attention so the Wp pool can reuse the freed
            # space and its DMAs overlap head-2 attention compute.
            with tc.tile_pool(name="inp", bufs=1) as inp, \
                 tc.tile_pool(name="wpool", bufs=8) as wpool:
                # ---------- Phase 0: load + transpose inputs ----------
                t1T = inp.tile([128, HC * L], F32R, name="t1T")
                t2T = inp.tile([128, HC * L], F32R, name="t2T")
                for src, dstT in ((t1, t1T), (t2, t2T)):
                    for c in range(LC):
                        nat = inp.tile([128, H], F32R, name="nat", tag="nat", bufs=3)
                        nc.sync.dma_start(out=nat[:], in_=src[c * 128:(c + 1) * 128, :])
                        for d in range(HC):
                            pt = psT.tile([128, 128], F32R, tag="tr")
                            nc.tensor.transpose(pt[:], nat[:, d * 128:(d + 1) * 128], ident[:])
                            nc.vector.tensor_copy(
                                dstT[:, d * L + c * 128: d * L + (c + 1) * 128], pt[:])

                projections(0, t1T, t2T, wpool)
                attention(0)
                projections(1, t1T, t2T, wpool)
                attention(1)
                projections(2, t1T, t2T, wpool)

            # ---------- head 2 attention + Phase 3: output projection ----------
            with tc.tile_pool(name="wp_pool", bufs=1) as wp_pool, \
                 tc.tile_pool(name="proj", bufs=2) as proj:
                wpt = wp_pool.tile([128, HET * H], F32R, name="wpt")
                for c in range(HET):
                    nc.sync.dma_start(out=wpt[:, c * H:(c + 1) * H],
                                      in_=wp[c * 128:(c + 1) * 128, :])
                attention(2)
                for qt in range(QT):
                    mi = proj.tile([128, HET * 128], F32R, name="mi", tag="mi")
                    nc.sync.dma_start(
                        out=mi[:].rearrange("p (h c q) -> p h c q", c=HC, q=128),
                        in_=mstash[:, :, :, qt * 128:(qt + 1) * 128]
                            .rearrange("h c p q -> p h c q"))
                    ob = proj.tile([128, H], F32, name="ob", tag="ob")
                    for (n0, nw) in ((0, 512), (512, 256)):
                        o_ps = psB.tile([128, nw], F32, tag="b")
                        for c in range(HET):
                            nc.tensor.matmul(
                                o_ps[:],
                                mi[:, c * 128:(c + 1) * 128],
                                wpt[:, c * H + n0: c * H + n0 + nw],
                                start=(c == 0), stop=(c == HET - 1))
                        nc.vector.tensor_add(ob[:, n0:n0 + nw], o_ps[:],
                                             bp_t[:, n0:n0 + nw])
                    nc.sync.dma_start(out=out_d[qt * 128:(qt + 1) * 128, :], in_=ob[:])

    nc.finalize()
    return nc


def kernel(tensor1, tensor2, Wq, bq, Wk, bk, Wv, bv, Wp, bp):
    from concourse.bass_utils import run_bass_kernel_spmd

    B = tensor1.shape[0]
    assert B == 8
    if "nc" not in _CACHE:
        _CACHE["nc"] = build()
    nc = _CACHE["nc"]

    f32 = np.float32
    shared = {
        "wq": np.ascontiguousarray(Wq, dtype=f32),
        "wk": np.ascontiguousarray(Wk, dtype=f32),
        "wv": np.ascontiguousarray(Wv, dtype=f32),
        "wp": np.ascontiguousarray(Wp, dtype=f32),
        "bq_sb": np.ascontiguousarray(
            bq.reshape(NH, HC, 128).transpose(0, 2, 1), dtype=f32),
        "bk_sb": np.ascontiguousarray(
            bk.reshape(NH, HC, 128).transpose(0, 2, 1), dtype=f32),
        "bv_bc": np.ascontiguousarray(
            np.broadcast_to(np.asarray(bv, dtype=f32)[:, None, :], (NH, 128, H))),
        "bp_bc": np.ascontiguousarray(
            np.broadcast_to(np.asarray(bp, dtype=f32)[None, :], (128, H))),
        "ident": np.eye(128, dtype=f32),
    }
    in_maps = [
        dict(shared,
             t1=np.ascontiguousarray(tensor1[b], dtype=f32),
             t2=np.ascontiguousarray(tensor2[b], dtype=f32))
        for b in range(B)
    ]
    res = run_bass_kernel_spmd(nc, in_maps, list(range(B)))
    return np.stack([res.results[b]["out"] for b in range(B)], axis=0)


# revision 8
# speedup vs baseline: 1.0889x; 1.0404x over previous
"""Trainium2 Bass kernel for nn_CrossAttention (3-head cross-attention + ReLU projection).

Sharding: data-parallel over batch. B=8 -> one batch element per NeuronCore,
identical SPMD program, no collectives. Full inputs in, full output out.

Per-core dataflow (all matmuls in fp32r: bf16 speed at free-dim>=256, ~1.5e-4 err):
  t1,t2 [1024,768]  --PE transpose-->  t1T,t2T [768,1024]
  per head h:
    qT = Wq_h^T-stationary matmuls over t1T (+bq fused in ACT eviction)   [768,1024]
    kT = same from t2T (+bk)                                              [768,1024]
    V  = t2T-stationary matmuls with Wv_h (+bv broadcast fused in DVE)    [1024,768]
    per q-tile (128 rows):
      S = qT^T @ kT  (PSUM, fp32)                [128,1024]
      softmax: reduce_max -> Exp(bias=-max, accum_out=rowsum) -> P (f32r, unnormalized)
      PT = PE-transpose(P)                       [1024,128]
      C = PT^T @ V  (PSUM)                       [128,768]
      multi = Relu(C * 1/rowsum)  (ACT eviction) -> PE transpose -> DRAM stash
  out = multiT^T-stationary @ Wp (+bp broadcast fused in DVE eviction)    [1024,768]
"""
import numpy as np

import concourse.bass as bass
import concourse.mybir as mybir
from concourse import bacc
from concourse.tile import TileContext

F32 = mybir.dt.float32
F32R = mybir.dt.float32r
AF = mybir.ActivationFunctionType
AX = mybir.AxisListType
ALU = mybir.AluOpType

L = 1024          # LQ = LK
H = 768           # H1 = H2
NH = 3            # heads
HC = H // 128     # 6 chunks of the hidden dim
LC = L // 128     # 8 chunks of the seq dim
QT = L // 128     # 8 q-tiles
HE = NH * H       # 2304 concat dim
HET = HE // 128   # 18 chunks

_CACHE = {}


def build():
    nc = bacc.Bacc()
    t1 = nc.declare_dram_parameter("t1", [L, H], F32R, isOutput=False)
    t2 = nc.declare_dram_parameter("t2", [L, H], F32R, isOutput=False)
    wq = nc.declare_dram_parameter("wq", [NH, H, H], F32R, isOutput=False)
    wk = nc.declare_dram_parameter("wk", [NH, H, H], F32R, isOutput=False)
    wv = nc.declare_dram_parameter("wv", [NH, H, H], F32R, isOutput=False)
    wp = nc.declare_dram_parameter("wp", [HE, H], F32R, isOutput=False)
    bq_sb = nc.declare_dram_parameter("bq_sb", [NH, 128, HC], F32, isOutput=False)
    bk_sb = nc.declare_dram_parameter("bk_sb", [NH, 128, HC], F32, isOutput=False)
    bv_bc = nc.declare_dram_parameter("bv_bc", [NH, 128, H], F32, isOutput=False)
    bp_bc = nc.declare_dram_parameter("bp_bc", [128, H], F32, isOutput=False)
    ident_d = nc.declare_dram_parameter("ident", [128, 128], F32R, isOutput=False)
    out_d = nc.declare_dram_parameter("out", [L, H], F32, isOutput=True)

    # DRAM stash for transposed relu(ctx): [head, e_chunk, 128, L]
    mstash = nc.dram_tensor("mstash", [NH, HC, 128, L], F32R)

    with TileContext(nc) as tc:
        with tc.tile_pool(name="psA", bufs=2, space="PSUM") as psA, \
             tc.tile_pool(name="psB", bufs=2, space="PSUM") as psB, \
             tc.tile_pool(name="psT", bufs=2, space="PSUM") as psT, \
             tc.tile_pool(name="small", bufs=1) as small, \
             tc.tile_pool(name="pers", bufs=1) as pers, \
             tc.tile_pool(name="work", bufs=2) as work, \
             tc.tile_pool(name="hb", bufs=2) as hb, \
             tc.tile_pool(name="stats", bufs=4) as stats:

            ident = small.tile([128, 128], F32R, name="ident")
            nc.sync.dma_start(out=ident[:], in_=ident_d[:])
            bp_t = small.tile([128, H], F32, name="bp_t")
            nc.sync.dma_start(out=bp_t[:], in_=bp_bc[:])

            qTt = pers.tile([128, HC * L], F32R, name="qTt")
            kTt = pers.tile([128, HC * L], F32R, name="kTt")
            Vt = pers.tile([128, LC * H], F32R, name="Vt")

            def projections(h, t1T, t2T, wpool, skip_q=False):
                bqs = hb.tile([128, HC], F32, name="bqs", tag="bqs")
                nc.sync.dma_start(out=bqs[:], in_=bq_sb[h])
                bks = hb.tile([128, HC], F32, name="bks", tag="bks")
                nc.sync.dma_start(out=bks[:], in_=bk_sb[h])
                bvb = hb.tile([128, H], F32, name="bvb", tag="bvb")
                nc.sync.dma_start(out=bvb[:], in_=bv_bc[h])

                pairs = ((wq, t1T, qTt, bqs), (wk, t2T, kTt, bks))
                if skip_q:
                    pairs = pairs[1:]
                for (wsrc, srcT, dstT, bias) in pairs:
                    wch = []
                    for d in range(HC):
                        wt = wpool.tile([128, H], F32R, name="w", tag="w")
                        nc.sync.dma_start(out=wt[:], in_=wsrc[h, d * 128:(d + 1) * 128, :])
                        wch.append(wt)
                    for e in range(HC):
                        for qh in range(2):
                            ps = psB.tile([128, 512], F32, tag="b")
                            for d in range(HC):
                                nc.tensor.matmul(
                                    ps[:],
                                    wch[d][:, e * 128:(e + 1) * 128],
                                    srcT[:, d * L + qh * 512: d * L + (qh + 1) * 512],
                                    start=(d == 0), stop=(d == HC - 1))
                            nc.scalar.activation(
                                dstT[:, e * L + qh * 512: e * L + (qh + 1) * 512],
                                ps[:], AF.Identity, bias=bias[:, e:e + 1], scale=1.0)

                wch = []
                for d in range(HC):
                    wt = wpool.tile([128, H], F32R, name="w", tag="w")
                    nc.sync.dma_start(out=wt[:], in_=wv[h, d * 128:(d + 1) * 128, :])
                    wch.append(wt)
                for kc in range(LC):
                    for (n0, nw) in ((0, 512), (512, 256)):
                        ps = psB.tile([128, nw], F32, tag="b")
                        for d in range(HC):
                            nc.tensor.matmul(
                                ps[:],
                                t2T[:, d * L + kc * 128: d * L + (kc + 1) * 128],
                                wch[d][:, n0:n0 + nw],
                                start=(d == 0), stop=(d == HC - 1))
                        nc.vector.tensor_add(
                            Vt[:, kc * H + n0: kc * H + n0 + nw],
                            ps[:], bvb[:, n0:n0 + nw])

            def attention(h, m2_pool=None):
                for qt in range(QT):
                    s_ps = psA.tile([128, 1024], F32, tag="s")
                    for nh2 in range(2):
                        for e in range(HC):
                            nc.tensor.matmul(
                                s_ps[:, nh2 * 512:(nh2 + 1) * 512],
                                qTt[:, e * L + qt * 128: e * L + (qt + 1) * 128],
                                kTt[:, e * L + nh2 * 512: e * L + (nh2 + 1) * 512],
                                start=(e == 0), stop=(e == HC - 1))
                    negmax = stats.tile([128, 1], F32, tag="nm")
                    nc.vector.tensor_reduce(negmax[:], s_ps[:], axis=AX.X,
                                            op=ALU.max, negate=True)
                    P = work.tile([128, 1024], F32R, name="P", tag="P")
                    esum = stats.tile([128, 1], F32, tag="es")
                    nc.scalar.activation(P[:], s_ps[:], AF.Exp,
                                         bias=negmax[:], scale=1.0, accum_out=esum[:])
                    rsum = stats.tile([128, 1], F32, tag="rs")
                    nc.vector.reciprocal(rsum[:], esum[:])

                    PT = work.tile([128, 1024], F32R, name="PT", tag="PT")
                    for kc in range(LC):
                        pt = psT.tile([128, 128], F32R, tag="tr")
                        nc.tensor.transpose(pt[:], P[:, kc * 128:(kc + 1) * 128], ident[:])
                        nc.vector.tensor_copy(PT[:, kc * 128:(kc + 1) * 128], pt[:])

                    mnat = work.tile([128, H], F32R, name="mnat", tag="mnat")
                    for (n0, nw) in ((0, 512), (512, 256)):
                        c_ps = psB.tile([128, nw], F32, tag="b")
                        for kc in range(LC):
                            nc.tensor.matmul(
                                c_ps[:],
                                PT[:, kc * 128:(kc + 1) * 128],
                                Vt[:, kc * H + n0: kc * H + n0 + nw],
                                start=(kc == 0), stop=(kc == LC - 1))
                        nc.scalar.activation(mnat[:, n0:n0 + nw], c_ps[:],
                                             AF.Relu, bias=0.0, scale=rsum[:])

                    if m2_pool is None:
                        mt = work.tile([128, H], F32R, name="mt", tag="mt")
                    else:
                        mt = m2_pool.tile([128, H], F32R, name="m2", tag="m2", bufs=QT)
                    for e in range(HC):
                        pt = psT.tile([128, 128], F32R, tag="tr")
                        nc.tensor.transpose(pt[:], mnat[:, e * 128:(e + 1) * 128], ident[:])
                        nc.vector.tensor_copy(mt[:, e * 128:(e + 1) * 128], pt[:])
                    if m2_pool is None:
                        nc.sync.dma_start(
                            out=mstash[h, :, :, qt * 128:(qt + 1) * 128]
                                .rearrange("c p q -> p c q"),
                            in_=mt[:].rearrange("p (c q) -> p c q", q=128))
                    else:
                        m2_tiles.append(mt)

            # Input transposes + all projections + attention for heads 0,1 live
            # inside the inputs/weights pool scope; the scope closes right
            # before head 2's attention so the Wp pool can reuse the freed
            # space and its DMAs overlap head-2 attention compute.
            with tc.tile_pool(name="inp", bufs=1) as inp, \
                 tc.tile_pool(name="wpool", bufs=8) as wpool:
                # ---------- Phase 0: load + transpose inputs ----------
                t1T = inp.tile([128, HC * L], F32R, name="t1T")
                t2T = inp.tile([128, HC * L], F32R, name="t2T")

                def transpose_in(srcd, dstT):
                    for c in range(LC):
                        nat = inp.tile([128, H], F32R, name="nat", tag="nat", bufs=3)
                        nc.sync.dma_start(out=nat[:], in_=srcd[c * 128:(c + 1) * 128, :])
                        for d in range(HC):
                            pt = psT.tile([128, 128], F32R, tag="tr")
                            nc.tensor.transpose(pt[:], nat[:, d * 128:(d + 1) * 128], ident[:])
                            nc.vector.tensor_copy(
                                dstT[:, d * L + c * 128: d * L + (c + 1) * 128], pt[:])

                # t1 first; head-0 q-projection matmuls then overlap t2's DMA
                transpose_in(t1, t1T)
                bqs0 = hb.tile([128, HC], F32, name="bqs", tag="bqs")
                nc.sync.dma_start(out=bqs0[:], in_=bq_sb[0])
                wch0 = []
                for d in range(HC):
                    wt = wpool.tile([128, H], F32R, name="w", tag="w")
                    nc.sync.dma_start(out=wt[:], in_=wq[0, d * 128:(d + 1) * 128, :])
                    wch0.append(wt)
                for e in range(HC):
                    for qh in range(2):
                        ps = psB.tile([128, 512], F32, tag="b")
                        for d in range(HC):
                            nc.tensor.matmul(
                                ps[:],
                                wch0[d][:, e * 128:(e + 1) * 128],
                                t1T[:, d * L + qh * 512: d * L + (qh + 1) * 512],
                                start=(d == 0), stop=(d == HC - 1))
                        nc.scalar.activation(
                            qTt[:, e * L + qh * 512: e * L + (qh + 1) * 512],
                            ps[:], AF.Identity, bias=bqs0[:, e:e + 1], scale=1.0)
                transpose_in(t2, t2T)

                projections(0, t1T, t2T, wpool, skip_q=True)
                attention(0)
                projections(1, t1T, t2T, wpool)
                attention(1)
                projections(2, t1T, t2T, wpool)

            # ---------- head 2 attention + Phase 3: output projection ----------
            with tc.tile_pool(name="wp_pool", bufs=1) as wp_pool, \
                 tc.tile_pool(name="m2", bufs=1) as m2_pool, \
                 tc.tile_pool(name="proj", bufs=2) as proj:
                wpt = wp_pool.tile([128, HET * H], F32R, name="wpt")
                for c in range(HET):
                    nc.sync.dma_start(out=wpt[:, c * H:(c + 1) * H],
                                      in_=wp[c * 128:(c + 1) * 128, :])
                m2_tiles = []
                attention(2, m2_pool=m2_pool)
                NH2 = (NH - 1) * HC  # chunks coming from the DRAM stash (heads 0,1)
                for qt in range(QT):
                    mi = proj.tile([128, NH2 * 128], F32R, name="mi", tag="mi")
                    nc.sync.dma_start(
                        out=mi[:].rearrange("p (h c q) -> p h c q", c=HC, q=128),
                        in_=mstash[:NH - 1, :, :, qt * 128:(qt + 1) * 128]
                            .rearrange("h c p q -> p h c q"))
                    ob = proj.tile([128, H], F32, name="ob", tag="ob")
                    for (n0, nw) in ((0, 512), (512, 256)):
                        o_ps = psB.tile([128, nw], F32, tag="b")
                        for c in range(HET):
                            if c < NH2:
                                lhs = mi[:, c * 128:(c + 1) * 128]
                            else:
                                lhs = m2_tiles[qt][:, (c - NH2) * 128:(c - NH2 + 1) * 128]
                            nc.tensor.matmul(
                                o_ps[:], lhs,
                                wpt[:, c * H + n0: c * H + n0 + nw],
                                start=(c == 0), stop=(c == HET - 1))
                        nc.vector.tensor_add(ob[:, n0:n0 + nw], o_ps[:],
                                             bp_t[:, n0:n0 + nw])
                    nc.sync.dma_start(out=out_d[qt * 128:(qt + 1) * 128, :], in_=ob[:])

    nc.finalize()
    return nc


def kernel(tensor1, tensor2, Wq, bq, Wk, bk, Wv, bv, Wp, bp):
    from concourse.bass_utils import run_bass_kernel_spmd

    B = tensor1.shape[0]
    assert B == 8
    if "nc" not in _CACHE:
        _CACHE["nc"] = build()
    nc = _CACHE["nc"]

    f32 = np.float32
    shared = {
        "wq": np.ascontiguousarray(Wq, dtype=f32),
        "wk": np.ascontiguousarray(Wk, dtype=f32),
        "wv": np.ascontiguousarray(Wv, dtype=f32),
        "wp": np.ascontiguousarray(Wp, dtype=f32),
        "bq_sb": np.ascontiguousarray(
            bq.reshape(NH, HC, 128).transpose(0, 2, 1), dtype=f32),
        "bk_sb": np.ascontiguousarray(
            bk.reshape(NH, HC, 128).transpose(0, 2, 1), dtype=f32),
        "bv_bc": np.ascontiguousarray(
            np.broadcast_to(np.asarray(bv, dtype=f32)[:, None, :], (NH, 128, H))),
        "bp_bc": np.ascontiguousarray(
            np.broadcast_to(np.asarray(bp, dtype=f32)[None, :], (128, H))),
        "ident": np.eye(128, dtype=f32),
    }
    in_maps = [
        dict(shared,
             t1=np.ascontiguousarray(tensor1[b], dtype=f32),
             t2=np.ascontiguousarray(tensor2[b], dtype=f32))
        for b in range(B)
    ]
    res = run_bass_kernel_spmd(nc, in_maps, list(range(B)))
    return np.stack([res.results[b]["out"] for b in range(B)], axis=0)
